# revision 1
# baseline (speedup 1.0000x reference)
"""GCNConv (gnn_message_passing) Trainium2 kernel — 8 NeuronCores, Bass/Tile.

Computes  out = segment_sum_dst(edge_vals * x[edge_src]) @ W + bias
for N=100000 nodes, E=3.2M edges, F=256, as fp32 in / fp32 out.

Strategy (all hardcoded for the 100000x256 / 3.2M-edge shape family):
  - Destination nodes are sharded over the 8 cores (12500 rows each); every
    core receives the full x in its HBM (staged by the runner, not kernel
    time) and produces its 12500 output rows; the host reassembles.
  - Per core, edges are grouped by (dst block of 128, src chunk of 25000)
    — the chunk split keeps gather indices within int16 range — sorted, and
    padded to 128-edge tiles.  The tile schedule is the max across cores so
    one SPMD program serves all 8 cores; per-core blocks are rank-matched by
    edge count to minimise the padding.
  - Per 128-edge tile, on device:
      msgs[e, f]  = dma_gather(x_chunk, src[e])          (SWDGE, 4 queues)
      A[e, d]     = vals[e] * (iota[d] == dstloc[e])     (DVE tensor_scalar)
      psum[d, f] += A.T @ msgs                           (PE, bf16 in / fp32 acc)
  - Per block epilogue: support -> PE transpose -> supportT.T @ W (fp32r)
    + bias -> 128 output rows.
  - x and A are bf16 (the gather traffic and the SBUF fabric are the
    bottleneck; measured end-to-end error vs the fp32 reference is ~2e-3
    of the output absmax).  The W matmul and bias run in fp32r/fp32.
"""
import os
import sys

sys.path.insert(0, '/opt/trn_rl_repo')
from contextlib import ExitStack

import ml_dtypes
import numpy as np

import concourse.bacc as bacc
import concourse.tile as tile
import concourse.mybir as mybir

F32 = mybir.dt.float32
F32R = mybir.dt.float32r
BF16 = mybir.dt.bfloat16
I16 = mybir.dt.int16
P = 128
GMAX = 8          # dma_gather limit: 1024 idxs (64 per Q7 lane)
AluOp = mybir.AluOpType

# ---- fixed problem/config constants -------------------------------------
N_NODES = 100000
F_IN = 256
F_OUT = 256
N_CORES = 8
CHUNK = 25000             # int16 gather-index range
NPC = N_NODES // N_CORES  # 12500 dst rows per core
NBLK = (NPC + P - 1) // P  # 98 blocks per core
CH = (N_NODES + CHUNK - 1) // CHUNK  # 4 src chunks
SB = 2                    # superblock: blocks whose gathers interleave
N_QUEUES = 4
MSG_BUFS = 6
A_BUFS = 6


def _make_schedule(counts_slot):
    """counts_slot: [NC, NBLK, CH] -> (tiles: block per global tile,
    chains: (chunk, first_tile, ntiles), order: (slot, ch, t))."""
    T_pos = np.ceil(counts_slot.max(axis=0) / P).astype(np.int64)
    tiles, chains, order = [], [], []
    for sb0 in range(0, NBLK, SB):
        blocks = range(sb0, min(sb0 + SB, NBLK))
        for ch in range(CH):
            run = []
            for b in blocks:
                for t in range(int(T_pos[b, ch])):
                    run.append((b, ch, t))
            i = 0
            while i < len(run):
                n = min(GMAX, len(run) - i)
                chains.append((ch, len(tiles) + i, n))
                i += n
            for (b, ch_, t) in run:
                tiles.append(b)
                order.append((b, ch_, t))
    return T_pos, tiles, chains, order, len(tiles)


def _preprocess(edge_src, edge_dst, edge_vals):
    edge_src = np.asarray(edge_src).astype(np.int64)
    edge_dst = np.asarray(edge_dst).astype(np.int64)
    edge_vals = np.asarray(edge_vals, dtype=np.float32)
    core = edge_dst // NPC
    dloc = edge_dst % NPC
    b = dloc // P
    j = dloc % P
    ch = edge_src // CHUNK
    s = edge_src % CHUNK

    gid = (core * NBLK + b) * CH + ch
    ngroups = N_CORES * NBLK * CH
    counts = np.bincount(gid, minlength=ngroups).reshape(N_CORES, NBLK, CH)
    # rank-match blocks to slots by per-core total count
    totals = counts.sum(axis=2)
    perms = np.argsort(-totals, axis=1, kind='stable')
    counts_slot = np.take_along_axis(counts, perms[:, :, None], axis=1)
    T_pos, tiles, chains, order, NT = _make_schedule(counts_slot)

    sort_idx = np.argsort(gid, kind='stable')
    s_sorted = s[sort_idx].astype(np.int16)
    j_sorted = j[sort_idx].astype(np.float32)
    v_sorted = edge_vals[sort_idx]
    group_starts = np.zeros(ngroups + 1, dtype=np.int64)
    np.cumsum(counts.reshape(-1), out=group_starts[1:])

    per_core = []
    for c in range(N_CORES):
        IDX = np.zeros((P, NT * 8), dtype=np.int16)
        DL = np.zeros((P, NT), dtype=np.float32)
        VL = np.zeros((P, NT), dtype=np.float32)
        for gt, (slot, cc, t) in enumerate(order):
            g = (c * NBLK + int(perms[c, slot])) * CH + cc
            e0, e1 = group_starts[g], group_starts[g + 1]
            cnt = int(e1 - e0)
            lo = t * P
            n = max(0, min(P, cnt - lo))
            sl = np.zeros(P, dtype=np.int16)
            if n > 0:
                sl[:n] = s_sorted[e0 + lo:e0 + lo + n]
                DL[:n, gt] = j_sorted[e0 + lo:e0 + lo + n]
                VL[:n, gt] = v_sorted[e0 + lo:e0 + lo + n]
            # gather idx i -> partition i%16, col gt*8 + i//16, replicated
            # to all 8 Q7-core slices of 16 partitions
            IDX[:, gt * 8:(gt + 1) * 8] = np.tile(sl.reshape(8, 16).T, (8, 1))
        per_core.append((IDX, DL, VL))
    return T_pos, tiles, chains, NT, per_core, perms


def _build_program(tiles, chains, NT):
    nc = bacc.Bacc("TRN2", debug=False, target_bir_lowering=False,
                   num_swdge_queues=N_QUEUES)
    x_d = nc.dram_tensor("x", [N_NODES, F_IN], BF16, kind="ExternalInput").ap()
    w_d = nc.dram_tensor("w", [F_IN, F_OUT], F32, kind="ExternalInput").ap()
    iota_d = nc.dram_tensor("iota", [P, P], BF16, kind="ExternalInput").ap()
    ident_d = nc.dram_tensor("ident", [P, P], F32, kind="ExternalInput").ap()
    biasb_d = nc.dram_tensor("biasb", [P, F_OUT], F32, kind="ExternalInput").ap()
    idx_d = nc.dram_tensor("idx", [P, NT * 8], I16, kind="ExternalInput").ap()
    dl_d = nc.dram_tensor("dstloc", [P, NT], F32, kind="ExternalInput").ap()
    vl_d = nc.dram_tensor("vals", [P, NT], F32, kind="ExternalInput").ap()
    out_d = nc.dram_tensor("out", [NBLK * P, F_OUT], F32,
                           kind="ExternalOutput").ap()
    KT = F_IN // P
    ntb = np.bincount(np.array(tiles), minlength=NBLK)

    with tile.TileContext(nc) as tc, ExitStack() as ctx:
        const = ctx.enter_context(tc.tile_pool(name="const", bufs=1))
        IDX = const.tile([P, NT * 8], I16)
        nc.sync.dma_start(IDX[:], idx_d[:])
        DL = const.tile([P, NT], F32)
        nc.sync.dma_start(DL[:], dl_d[:])
        VL = const.tile([P, NT], F32)
        nc.sync.dma_start(VL[:], vl_d[:])
        IOTA = const.tile([P, P], BF16)
        nc.sync.dma_start(IOTA[:], iota_d[:])
        IDENT = const.tile([P, P], F32R)
        nc.sync.dma_start(IDENT[:], ident_d[:].bitcast(F32R))
        BIASB = const.tile([P, F_OUT], F32)
        nc.sync.dma_start(BIASB[:], biasb_d[:])
        Wt = []
        for k in range(KT):
            wk = const.tile([P, F_OUT], F32R, tag=f"w{k}", name=f"w{k}")
            nc.sync.dma_start(wk[:], w_d[k * P:(k + 1) * P, :].bitcast(F32R))
            Wt.append(wk)

        gp = ctx.enter_context(tc.tile_pool(name="msgs", bufs=MSG_BUFS))
        apool = ctx.enter_context(tc.tile_pool(name="atile", bufs=A_BUFS))
        ep = ctx.enter_context(tc.tile_pool(name="epil", bufs=2))
        ps_s = ctx.enter_context(
            tc.tile_pool(name="ps_s", bufs=SB + 1, space="PSUM"))
        ps_t = ctx.enter_context(tc.tile_pool(name="ps_t", bufs=2, space="PSUM"))
        ps_o = ctx.enter_context(tc.tile_pool(name="ps_o", bufs=2, space="PSUM"))

        psum_of = {}
        mm_count = {}
        chain_i = 0
        gath_q = 0

        def epilogue(b):
            psum_s = psum_of.pop(b)
            s_sb = ep.tile([P, F_IN], F32R, tag="s_sb", name="s_sb")
            nc.scalar.copy(s_sb[:], psum_s[:])
            outp = ps_o.tile([P, F_OUT], F32, tag="outp", name="outp")
            for h in range(KT):
                pt = ps_t.tile([P, P], F32R, tag="pt", name="pt")
                nc.tensor.transpose(pt[:], s_sb[:, h * P:(h + 1) * P], IDENT[:])
                sth = ep.tile([P, P], F32R, tag="sth", name="sth")
                nc.scalar.copy(sth[:], pt[:])
                nc.tensor.matmul(outp[:], sth[:], Wt[h][:],
                                 start=(h == 0), stop=(h == KT - 1))
            ob = ep.tile([P, F_OUT], F32, tag="ob", name="ob")
            nc.vector.tensor_tensor(ob[:], outp[:], BIASB[:], op=AluOp.add)
            nc.sync.dma_start(out_d[b * P:(b + 1) * P, :], ob[:])

        for sb0 in range(0, NBLK, SB):
            sb_blocks = list(range(sb0, min(sb0 + SB, NBLK)))
            while chain_i < len(chains):
                ch, t0, n = chains[chain_i]
                if tiles[t0] not in sb_blocks:
                    break
                chain_i += 1
                hi = min((ch + 1) * CHUNK, N_NODES)
                g = gp.tile([P, n, F_IN], BF16, tag="msgs", name="msgs")
                nc.gpsimd.dma_gather(
                    g[:], x_d[ch * CHUNK:hi, :], IDX[:, t0 * 8:(t0 + n) * 8],
                    n * P, n * P, F_IN, queue_num=gath_q % N_QUEUES,
                )
                gath_q += 1
                for t in range(n):
                    gt = t0 + t
                    b = tiles[gt]
                    if b not in psum_of:
                        psum_of[b] = ps_s.tile([P, F_IN], F32, tag="psum_s",
                                               name=f"psum_s{b}")
                        mm_count[b] = 0
                    A = apool.tile([P, P], BF16, tag="atile", name="atile")
                    nc.vector.tensor_scalar(
                        A[:], IOTA[:], DL[:, gt:gt + 1], VL[:, gt:gt + 1],
                        AluOp.is_equal, AluOp.mult)
                    nc.tensor.matmul(
                        psum_of[b][:], A[:], g[:, t, :],
                        start=(mm_count[b] == 0),
                        stop=(mm_count[b] == int(ntb[b]) - 1))
                    mm_count[b] += 1
            for b in sb_blocks:
                epilogue(b)
        assert chain_i == len(chains)

    nc.compile()
    return nc


def _install_profile_shim():
    """antenv.axon_hooks is absent in this image; recreate it so
    run_bass_kernel_spmd(trace=True) can NTFF-profile under axon."""
    import types
    if "antenv.axon_hooks" in sys.modules:
        return
    import antenv
    mod = types.ModuleType("antenv.axon_hooks")
    mod._hook = None

    def set_axon_ntff_profile_hook(h):
        mod._hook = h

    def get_axon_ntff_profile_hook():
        if mod._hook is None:
            try:
                from trn_agent_boot.trn_boot import _ntff_profile_via_ctypes
                mod._hook = _ntff_profile_via_ctypes('/opt/axon/libaxon_pjrt.so')
            except Exception:
                return None
        return mod._hook

    mod.set_axon_ntff_profile_hook = set_axon_ntff_profile_hook
    mod.get_axon_ntff_profile_hook = get_axon_ntff_profile_hook
    sys.modules["antenv.axon_hooks"] = mod
    antenv.axon_hooks = mod


_PROGRAM_CACHE = {}


def kernel(x, edge_src, edge_dst, edge_vals, W, bias):
    x = np.asarray(x, dtype=np.float32)
    W = np.asarray(W, dtype=np.float32)
    bias = np.asarray(bias, dtype=np.float32)
    assert x.shape == (N_NODES, F_IN), x.shape

    T_pos, tiles, chains, NT, per_core, perms = _preprocess(
        edge_src, edge_dst, edge_vals)

    key = (NT, tuple(tiles), tuple(chains))
    if key not in _PROGRAM_CACHE:
        _PROGRAM_CACHE.clear()
        _PROGRAM_CACHE[key] = _build_program(tiles, chains, NT)
    nc = _PROGRAM_CACHE[key]

    x_bf = x.astype(ml_dtypes.bfloat16)
    iota = np.broadcast_to(np.arange(P).astype(ml_dtypes.bfloat16),
                           (P, P)).copy()
    ident = np.eye(P, dtype=np.float32)
    biasb = np.broadcast_to(bias, (P, F_OUT)).copy()
    maps = []
    for c in range(N_CORES):
        IDX, DL, VL = per_core[c]
        maps.append({"x": x_bf, "w": W, "iota": iota, "ident": ident,
                     "biasb": biasb, "idx": IDX, "dstloc": DL, "vals": VL})

    trace = os.environ.get("GCN_KERNEL_TRACE", "0") == "1"
    if trace:
        _install_profile_shim()
    from concourse.bass_utils import run_bass_kernel_spmd
    res = run_bass_kernel_spmd(nc, maps, list(range(N_CORES)), trace=trace)
    if trace and res.exec_time_ns is not None:
        print(f"HW exec time: {res.exec_time_ns} ns")

    out = np.empty((N_NODES, F_OUT), dtype=np.float32)
    for c in range(N_CORES):
        r = res.results[c]["out"]
        for s in range(NBLK):
            blk = int(perms[c, s])
            rows = min(P, NPC - blk * P)
            out[c * NPC + blk * P: c * NPC + blk * P + rows, :] = \
                r[s * P: s * P + rows, :]
    return out



# revision 8
# speedup vs baseline: 1.4386x; 1.4386x over previous
"""GCNConv (gnn_message_passing) Trainium2 kernel — 8 NeuronCores, Bass/Tile.

Computes  out = segment_sum_dst(edge_vals * x[edge_src]) @ W + bias
for N=100000 nodes, E=3.2M edges, F=256, as fp32 in / fp32 out.

Strategy (all hardcoded for the 100000x256 / 3.2M-edge shape family):
  - Destination nodes are sharded over the 8 cores (12500 rows each); every
    core receives the full x in its HBM (staged by the runner, not kernel
    time) and produces its 12500 output rows; the host reassembles.
  - Per core, edges are grouped by (dst block of 128, src chunk of 25000)
    — the chunk split keeps gather indices within int16 range — sorted, and
    padded to 128-edge tiles.  The tile schedule is the max across cores so
    one SPMD program serves all 8 cores; per-core blocks are rank-matched by
    edge count to minimise the padding.
  - Per 128-edge tile, on device:
      msgs[e, f]  = dma_gather(x_chunk, src[e])          (SWDGE, 4 queues)
      A[e, d]     = vals[e] * (iota[d] == dstloc[e])     (DVE tensor_scalar)
      psum[d, f] += A.T @ msgs                           (PE, bf16 in / fp32 acc)
  - Per block epilogue: support -> PE transpose -> supportT.T @ W (fp32r)
    + bias -> 128 output rows.
  - x and A are bf16 (the gather traffic and the SBUF fabric are the
    bottleneck; measured end-to-end error vs the fp32 reference is ~2e-3
    of the output absmax).  The W matmul and bias run in fp32r/fp32.
"""
import os
import sys

sys.path.insert(0, '/opt/trn_rl_repo')
from contextlib import ExitStack

import ml_dtypes
import numpy as np

import concourse.bacc as bacc
import concourse.tile as tile
import concourse.mybir as mybir

F32 = mybir.dt.float32
F32R = mybir.dt.float32r
BF16 = mybir.dt.bfloat16
I16 = mybir.dt.int16
P = 128
GMAX = 8          # dma_gather ucode limit: 1024 idxs (64 per Q7 lane)
AluOp = mybir.AluOpType

# ---- fixed problem/config constants -------------------------------------
N_NODES = 100000
F_IN = 256
F_OUT = 256
N_CORES = 8
CHUNK = 25000             # int16 gather-index range
NPC = N_NODES // N_CORES  # 12500 dst rows per core
NBLK = (NPC + P - 1) // P  # 98 blocks per core
CH = (N_NODES + CHUNK - 1) // CHUNK  # 4 src chunks
SB = 3                    # superblock: blocks whose gathers interleave
N_QUEUES = 4
MSG_BUFS = 6
A_BUFS = 4


def _make_schedule(counts_slot):
    """counts_slot: [NC, NBLK, CH] -> (tiles: block per global tile,
    chains: (chunk, first_tile, ntiles), order: (slot, ch, t))."""
    T_pos = np.ceil(counts_slot.max(axis=0) / P).astype(np.int64)
    tiles, chains, order = [], [], []
    for sb0 in range(0, NBLK, SB):
        blocks = range(sb0, min(sb0 + SB, NBLK))
        for ch in range(CH):
            run = []
            for b in blocks:
                for t in range(int(T_pos[b, ch])):
                    run.append((b, ch, t))
            i = 0
            while i < len(run):
                n = min(GMAX, len(run) - i)
                chains.append((ch, len(tiles) + i, n))
                i += n
            for (b, ch_, t) in run:
                tiles.append(b)
                order.append((b, ch_, t))
    return T_pos, tiles, chains, order, len(tiles)


def _preprocess(edge_src, edge_dst, edge_vals):
    edge_src = np.asarray(edge_src).astype(np.int64)
    edge_dst = np.asarray(edge_dst).astype(np.int64)
    edge_vals = np.asarray(edge_vals, dtype=np.float32)
    core = edge_dst // NPC
    dloc = edge_dst % NPC
    b = dloc // P
    j = dloc % P
    ch = edge_src // CHUNK
    s = edge_src % CHUNK

    gid = (core * NBLK + b) * CH + ch
    ngroups = N_CORES * NBLK * CH
    counts = np.bincount(gid, minlength=ngroups).reshape(N_CORES, NBLK, CH)
    # rank-match blocks to slots by per-core total count
    totals = counts.sum(axis=2)
    perms = np.argsort(-totals, axis=1, kind='stable')
    counts_slot = np.take_along_axis(counts, perms[:, :, None], axis=1)
    T_pos, tiles, chains, order, NT = _make_schedule(counts_slot)

    sort_idx = np.argsort(gid, kind='stable')
    s_sorted = s[sort_idx].astype(np.int16)
    j_sorted = j[sort_idx].astype(np.float32)
    v_sorted = edge_vals[sort_idx]
    group_starts = np.zeros(ngroups + 1, dtype=np.int64)
    np.cumsum(counts.reshape(-1), out=group_starts[1:])

    per_core = []
    for c in range(N_CORES):
        IDX = np.zeros((P, NT * 8), dtype=np.int16)
        DL = np.zeros((P, NT), dtype=ml_dtypes.bfloat16)
        VL = np.zeros((P, NT), dtype=ml_dtypes.bfloat16)
        for gt, (slot, cc, t) in enumerate(order):
            g = (c * NBLK + int(perms[c, slot])) * CH + cc
            e0, e1 = group_starts[g], group_starts[g + 1]
            cnt = int(e1 - e0)
            lo = t * P
            n = max(0, min(P, cnt - lo))
            sl = np.zeros(P, dtype=np.int16)
            if n > 0:
                sl[:n] = s_sorted[e0 + lo:e0 + lo + n]
                DL[:n, gt] = j_sorted[e0 + lo:e0 + lo + n]
                VL[:n, gt] = v_sorted[e0 + lo:e0 + lo + n]
            # gather idx i -> partition i%16, col gt*8 + i//16, replicated
            # to all 8 Q7-core slices of 16 partitions
            IDX[:, gt * 8:(gt + 1) * 8] = np.tile(sl.reshape(8, 16).T, (8, 1))
        per_core.append((IDX, DL, VL))
    return T_pos, tiles, chains, NT, per_core, perms


def _build_program(tiles, chains, NT):
    nc = bacc.Bacc("TRN2", debug=False, target_bir_lowering=False,
                   num_swdge_queues=N_QUEUES)
    x_d = nc.dram_tensor("x", [N_NODES, F_IN], BF16, kind="ExternalInput").ap()
    w_d = nc.dram_tensor("w", [F_IN, F_OUT], F32, kind="ExternalInput").ap()
    iota_d = nc.dram_tensor("iota", [P, P], BF16, kind="ExternalInput").ap()
    ident_d = nc.dram_tensor("ident", [P, P], F32, kind="ExternalInput").ap()
    biasb_d = nc.dram_tensor("biasb", [P, F_OUT], F32, kind="ExternalInput").ap()
    idx_d = nc.dram_tensor("idx", [P, NT * 8], I16, kind="ExternalInput").ap()
    dl_d = nc.dram_tensor("dstloc", [P, NT], BF16, kind="ExternalInput").ap()
    vl_d = nc.dram_tensor("vals", [P, NT], BF16, kind="ExternalInput").ap()
    out_d = nc.dram_tensor("out", [NBLK * P, F_OUT], F32,
                           kind="ExternalOutput").ap()
    KT = F_IN // P
    ntb = np.bincount(np.array(tiles), minlength=NBLK)

    with tile.TileContext(nc) as tc, ExitStack() as ctx:
        const = ctx.enter_context(tc.tile_pool(name="const", bufs=1))
        IDX = const.tile([P, NT * 8], I16)
        nc.sync.dma_start(IDX[:], idx_d[:])
        DL = const.tile([P, NT], BF16)
        nc.sync.dma_start(DL[:], dl_d[:])
        VL = const.tile([P, NT], BF16)
        nc.sync.dma_start(VL[:], vl_d[:])
        IOTA = const.tile([P, P], BF16)
        nc.sync.dma_start(IOTA[:], iota_d[:])
        IDENT = const.tile([P, P], F32R)
        nc.sync.dma_start(IDENT[:], ident_d[:].bitcast(F32R))
        BIASB = const.tile([P, F_OUT], F32)
        nc.sync.dma_start(BIASB[:], biasb_d[:])
        Wt = []
        for k in range(KT):
            wk = const.tile([P, F_OUT], F32R, tag=f"w{k}", name=f"w{k}")
            nc.sync.dma_start(wk[:], w_d[k * P:(k + 1) * P, :].bitcast(F32R))
            Wt.append(wk)

        gp = ctx.enter_context(tc.tile_pool(name="msgs", bufs=MSG_BUFS))
        apool = ctx.enter_context(tc.tile_pool(name="atile", bufs=A_BUFS))
        ep = ctx.enter_context(tc.tile_pool(name="epil", bufs=2))
        ps_s = ctx.enter_context(
            tc.tile_pool(name="ps_s", bufs=SB + 1, space="PSUM"))
        ps_t = ctx.enter_context(tc.tile_pool(name="ps_t", bufs=2, space="PSUM"))
        ps_o = ctx.enter_context(tc.tile_pool(name="ps_o", bufs=2, space="PSUM"))

        psum_of = {}
        mm_count = {}
        chain_i = 0
        gath_q = 0

        def epilogue(b):
            psum_s = psum_of.pop(b)
            s_sb = ep.tile([P, F_IN], F32R, tag="s_sb", name="s_sb")
            nc.scalar.copy(s_sb[:], psum_s[:])
            outp = ps_o.tile([P, F_OUT], F32, tag="outp", name="outp")
            for h in range(KT):
                pt = ps_t.tile([P, P], F32R, tag="pt", name="pt")
                nc.tensor.transpose(pt[:], s_sb[:, h * P:(h + 1) * P], IDENT[:])
                sth = ep.tile([P, P], F32R, tag="sth", name="sth")
                nc.scalar.copy(sth[:], pt[:])
                nc.tensor.matmul(outp[:], sth[:], Wt[h][:],
                                 start=(h == 0), stop=(h == KT - 1))
            ob = ep.tile([P, F_OUT], F32, tag="ob", name="ob")
            nc.vector.tensor_tensor(ob[:], outp[:], BIASB[:], op=AluOp.add)
            nc.sync.dma_start(out_d[b * P:(b + 1) * P, :], ob[:])

        for sb0 in range(0, NBLK, SB):
            sb_blocks = list(range(sb0, min(sb0 + SB, NBLK)))
            while chain_i < len(chains):
                ch, t0, n = chains[chain_i]
                if tiles[t0] not in sb_blocks:
                    break
                chain_i += 1
                hi = min((ch + 1) * CHUNK, N_NODES)
                g = gp.tile([P, n, F_IN], BF16, tag="msgs", name="msgs")
                nc.gpsimd.dma_gather(
                    g[:], x_d[ch * CHUNK:hi, :], IDX[:, t0 * 8:(t0 + n) * 8],
                    n * P, n * P, F_IN, queue_num=gath_q % N_QUEUES,
                )
                gath_q += 1
                t1 = apool.tile([P, n, P], BF16, tag="t1", name="t1")
                ab = apool.tile([P, n, P], BF16, tag="ab", name="ab")
                nc.vector.tensor_tensor(
                    t1[:], IOTA[:].unsqueeze(1).broadcast_to([P, n, P]),
                    DL[:, t0:t0 + n].unsqueeze(2).broadcast_to([P, n, P]),
                    op=AluOp.is_equal)
                nc.vector.tensor_tensor(
                    ab[:], t1[:],
                    VL[:, t0:t0 + n].unsqueeze(2).broadcast_to([P, n, P]),
                    op=AluOp.mult)
                for t in range(n):
                    gt = t0 + t
                    b = tiles[gt]
                    if b not in psum_of:
                        psum_of[b] = ps_s.tile([P, F_IN], F32, tag="psum_s",
                                               name=f"psum_s{b}")
                        mm_count[b] = 0
                    nc.tensor.matmul(
                        psum_of[b][:], ab[:, t, :], g[:, t, :],
                        start=(mm_count[b] == 0),
                        stop=(mm_count[b] == int(ntb[b]) - 1))
                    mm_count[b] += 1
            for b in sb_blocks:
                epilogue(b)
        assert chain_i == len(chains)

    nc.compile()
    return nc


def _install_profile_shim():
    """antenv.axon_hooks is absent in this image; recreate it so
    run_bass_kernel_spmd(trace=True) can NTFF-profile under axon."""
    import types
    if "antenv.axon_hooks" in sys.modules:
        return
    import antenv
    mod = types.ModuleType("antenv.axon_hooks")
    mod._hook = None

    def set_axon_ntff_profile_hook(h):
        mod._hook = h

    def get_axon_ntff_profile_hook():
        if mod._hook is None:
            try:
                from trn_agent_boot.trn_boot import _ntff_profile_via_ctypes
                mod._hook = _ntff_profile_via_ctypes('/opt/axon/libaxon_pjrt.so')
            except Exception:
                return None
        return mod._hook

    mod.set_axon_ntff_profile_hook = set_axon_ntff_profile_hook
    mod.get_axon_ntff_profile_hook = get_axon_ntff_profile_hook
    sys.modules["antenv.axon_hooks"] = mod
    antenv.axon_hooks = mod


_PROGRAM_CACHE = {}


def kernel(x, edge_src, edge_dst, edge_vals, W, bias):
    x = np.asarray(x, dtype=np.float32)
    W = np.asarray(W, dtype=np.float32)
    bias = np.asarray(bias, dtype=np.float32)
    assert x.shape == (N_NODES, F_IN), x.shape

    T_pos, tiles, chains, NT, per_core, perms = _preprocess(
        edge_src, edge_dst, edge_vals)

    key = (NT, tuple(tiles), tuple(chains))
    if key not in _PROGRAM_CACHE:
        _PROGRAM_CACHE.clear()
        _PROGRAM_CACHE[key] = _build_program(tiles, chains, NT)
    nc = _PROGRAM_CACHE[key]

    x_bf = x.astype(ml_dtypes.bfloat16)
    iota = np.broadcast_to(np.arange(P).astype(ml_dtypes.bfloat16),
                           (P, P)).copy()
    ident = np.eye(P, dtype=np.float32)
    biasb = np.broadcast_to(bias, (P, F_OUT)).copy()
    maps = []
    for c in range(N_CORES):
        IDX, DL, VL = per_core[c]
        maps.append({"x": x_bf, "w": W, "iota": iota, "ident": ident,
                     "biasb": biasb, "idx": IDX, "dstloc": DL, "vals": VL})

    trace = os.environ.get("GCN_KERNEL_TRACE", "0") == "1"
    if trace:
        _install_profile_shim()
    from concourse.bass_utils import run_bass_kernel_spmd
    res = run_bass_kernel_spmd(nc, maps, list(range(N_CORES)), trace=trace)
    if trace and res.exec_time_ns is not None:
        print(f"HW exec time: {res.exec_time_ns} ns")

    out = np.empty((N_NODES, F_OUT), dtype=np.float32)
    for c in range(N_CORES):
        r = res.results[c]["out"]
        for s in range(NBLK):
            blk = int(perms[c, s])
            rows = min(P, NPC - blk * P)
            out[c * NPC + blk * P: c * NPC + blk * P + rows, :] = \
                r[s * P: s * P + rows, :]
    return out

